# revision 27
# baseline (speedup 1.0000x reference)
"""Grouped-expert SwiGLU (MoE) kernel for Trainium2, expert-parallel over 8 cores.

Per core (one expert):
    g = x @ W_gate          [T, DOUT]
    u = x @ W_down          [T, DOUT]
    h = silu(g) * u
    out = h @ W_up          [T, DIN]

All inputs are pre-cast to bf16 and pre-laid-out on the host so the device
does no transposes and no input casts — the PE runs a dense LDW+MM stream at
the bf16 roofline (~216 ns per [128x128]x[128x512] matmul):
  x_t    [S1, KC, P, NS]  xT chunks: x_t[s,k,p,n] = x[s*NS+n, k*P+p]
  gate_t [JC, P, DIN]     per-j panels: gate_t[j,p,k*P+n] = Wg[k*P+p, j*P+n]
  down_t [JC, P, DIN]     same layout as gate_t
  up_t   [JC, P, DIN]     up_t[j,p,c] = Wu[j*P+p, c]
phase 1: hT[j] = silu(Wg[:,j].T @ xT) * (Wd[:,j].T @ xT)   [dout, tokens]
phase 2: out[m,:] = sum_j hT[j][:,m].T @ Wu[j,:]           [tokens, din]
Matmuls in bf16 with fp32 PSUM accumulation.

DMA notes (measured): each DMA_DIRECT2D dispatch costs ~650ns serialized on
its ring (Sync/Scalar are the two HWDGE rings); a single in-flight transfer
streams ~100GB/s, concurrent transfers ~400GB/s aggregate.  So the j0 window
emits DMAs in exact consumption order, finest pieces first, across both
rings; later strips use big transfers (fewer dispatches).
"""

import numpy as np
import ml_dtypes

import concourse.bacc as bacc
import concourse.mybir as mybir
from concourse.tile import TileContext
from concourse.bass_utils import run_bass_kernel_spmd

F32 = mybir.dt.float32
BF16 = mybir.dt.bfloat16
SILU = mybir.ActivationFunctionType.Silu
SIGMOID = mybir.ActivationFunctionType.Sigmoid
COPY = mybir.ActivationFunctionType.Copy

E = 8
T, DIN, DOUT = 2048, 2048, 1408
P = 128
NS = 512
KC = DIN // P   # 16 contraction chunks (din)
JC = DOUT // P  # 11 dout blocks
MC = T // P     # 16 token blocks
S1 = T // NS    # 4 token strips
S2 = DIN // NS  # 4 din strips


def build_program(sim_safe=False):
    nc = bacc.Bacc(target_bir_lowering=False, trn_type="TRN2")
    xt = nc.dram_tensor("x_t", [S1, KC, P, NS], BF16, kind="ExternalInput")
    wg = nc.dram_tensor("gate_t", [JC, P, DIN], BF16, kind="ExternalInput")
    wd = nc.dram_tensor("down_t", [JC, P, DIN], BF16, kind="ExternalInput")
    wu = nc.dram_tensor("up_t", [JC, P, DIN], BF16, kind="ExternalInput")
    out = nc.dram_tensor("out", [T, DIN], F32, kind="ExternalOutput")

    with TileContext(nc) as tc:
        with tc.tile_pool(name="persist", bufs=1) as persist:
            xts0 = [persist.tile([P, 2, NS], BF16, tag=f"xt0_{c}",
                                 name=f"xt0_{c}")
                    for c in range(KC // 2)]
            xts3 = [persist.tile([P, KC, NS], BF16, tag=f"xt3_{s}",
                                 name=f"xt3_{s}")
                    for s in range(1, S1)]
            hT = [persist.tile([P, T], BF16, tag=f"hT{j}", name=f"hT{j}")
                  for j in range(JC)]
            wub = [persist.tile([P, DIN], BF16, tag=f"wub{j}", name=f"wub{j}")
                   for j in range(JC)]

            with tc.tile_pool(name="wstage", bufs=2) as wstage, \
                 tc.tile_pool(name="silu", bufs=3) as silu_pool, \
                 tc.tile_pool(name="ostage", bufs=4) as ostage, \
                 tc.tile_pool(name="p1", bufs=2, space="PSUM") as p1, \
                 tc.tile_pool(name="p2", bufs=4, space="PSUM") as p2:

                def x_rhs(s, k):
                    if s == 0:
                        return xts0[k // 2][:, k % 2, :]
                    return xts3[s - 1][:, k, :]

                # ---- phase 1: hT[j] = silu(gT) * uT ----
                for j in range(JC):
                    wgp = wstage.tile([P, DIN], BF16, tag="wgp", name=f"wgp{j}")
                    wdp = wstage.tile([P, DIN], BF16, tag="wdp", name=f"wdp{j}")
                    if j == 0:
                        # j0 panels in 128KB pieces so the first matmul only
                        # waits on a piece of panel + one x chunk
                        for wp, wdr in ((wgp, wg), (wdp, wd)):
                            for q in range(4):
                                cols = slice(4 * q * P, 4 * (q + 1) * P)
                                nc.sync.dma_start(out=wp[:, cols],
                                                  in_=wdr.ap()[0][:, cols])
                        # strip 0: first pair as two 128KB singles, then
                        # 256KB chunks on the Scalar ring; the last three
                        # chunks go on the Sync ring (ahead of the strip
                        # quarters) so both rings finish strip 0 early
                        for k in range(2):
                            nc.scalar.dma_start(
                                out=xts0[0][:, k:k + 1, :],
                                in_=xt.ap()[0, k:k + 1]
                                .rearrange("k p n -> p k n"))
                        for c in range(1, KC // 2):
                            ring = nc.scalar if c < 5 else nc.sync
                            ring.dma_start(
                                out=xts0[c],
                                in_=xt.ap()[0, 2 * c:2 * c + 2]
                                .rearrange("k p n -> p k n"))
                        # strips 1-3 in 512KB quarters on the Sync ring
                        for s in range(1, S1):
                            for q in range(4):
                                nc.sync.dma_start(
                                    out=xts3[s - 1][:, 4 * q:4 * q + 4, :],
                                    in_=xt.ap()[s, 4 * q:4 * q + 4]
                                    .rearrange("k p n -> p k n"))
                    elif j == 1:
                        # j1 panels ride the otherwise-idle Scalar ring so
                        # the Sync ring stays dedicated to x strips
                        nc.scalar.dma_start(out=wgp, in_=wg.ap()[j])
                        nc.scalar.dma_start(out=wdp, in_=wd.ap()[j])
                    else:
                        nc.sync.dma_start(out=wgp, in_=wg.ap()[j])
                        nc.sync.dma_start(out=wdp, in_=wd.ap()[j])
                    for s in range(S1):
                        pg = p1.tile([P, NS], F32, tag="pg", name="pg")
                        pu = p1.tile([P, NS], F32, tag="pu", name="pu")
                        # interleave gate/down per k: halves the x-chunk
                        # consumption rate so DMA supply keeps up during j0
                        for k in range(KC):
                            nc.tensor.matmul(
                                pg, lhsT=wgp[:, k * P:(k + 1) * P],
                                rhs=x_rhs(s, k),
                                start=(k == 0), stop=(k == KC - 1))
                            nc.tensor.matmul(
                                pu, lhsT=wdp[:, k * P:(k + 1) * P],
                                rhs=x_rhs(s, k),
                                start=(k == 0), stop=(k == KC - 1))
                        sl = silu_pool.tile([P, NS], BF16, tag="sl", name="sl")
                        if sim_safe:
                            # CoreSim has no Silu; silu(g) = g * sigmoid(g)
                            nc.scalar.activation(sl, pg, SIGMOID)
                            nc.vector.tensor_mul(out=sl, in0=sl, in1=pg)
                        else:
                            nc.scalar.activation(sl, pg, SILU)
                        nc.vector.tensor_mul(out=hT[j][:, s * NS:(s + 1) * NS],
                                             in0=sl, in1=pu)

                # stage phase-2 weights; the Sync ring reaches these right
                # after the phase-1 panels, well before phase 2 needs them
                for j in range(JC):
                    nc.sync.dma_start(out=wub[j], in_=wu.ap()[j])

                # ---- phase 2: out = hT.T @ Wu ----
                for m in range(MC):
                    for n in range(S2):
                        msl = slice(m * P, (m + 1) * P)
                        if m == MC - 1 and n == S2 - 1:
                            # final group as two N=256 halves: the first
                            # half's evict+DMA overlaps the second half's
                            # matmuls, shortening the kernel tail
                            H = NS // 2
                            for h in range(2):
                                dsl = slice(n * NS + h * H,
                                            n * NS + (h + 1) * H)
                                po = p2.tile([P, H], F32, tag="po",
                                             name=f"poL{h}")
                                for j in range(JC):
                                    nc.tensor.matmul(
                                        po, lhsT=hT[j][:, msl],
                                        rhs=wub[j][:, dsl],
                                        start=(j == 0), stop=(j == JC - 1))
                                ot = ostage.tile([P, H], F32, tag="ot",
                                                 name=f"oL{h}")
                                if h == 0:
                                    nc.vector.tensor_copy(out=ot, in_=po)
                                    nc.sync.dma_start(
                                        out=out.ap()[msl, dsl], in_=ot)
                                else:
                                    nc.scalar.activation(ot, po, COPY)
                                    nc.scalar.dma_start(
                                        out=out.ap()[msl, dsl], in_=ot)
                            continue
                        dsl = slice(n * NS, (n + 1) * NS)
                        po = p2.tile([P, NS], F32, tag="po", name="po")
                        for j in range(JC):
                            nc.tensor.matmul(
                                po, lhsT=hT[j][:, msl],
                                rhs=wub[j][:, dsl],
                                start=(j == 0), stop=(j == JC - 1))
                        ot = ostage.tile([P, NS], F32, tag="ot", name="ot")
                        if (m * S2 + n) % 2 == 0:
                            nc.scalar.activation(ot, po, COPY)
                        else:
                            nc.vector.tensor_copy(out=ot, in_=po)
                        nc.sync.dma_start(
                            out=out.ap()[msl, dsl], in_=ot)

    nc.finalize()
    return nc


_BF = ml_dtypes.bfloat16


def make_in_maps(x, gate_proj, down_proj, up_proj):
    maps = []
    for e in range(E):
        xtb = x[e].T.astype(_BF)  # [DIN, T]
        xtb = np.ascontiguousarray(
            xtb.reshape(KC, P, S1, NS).transpose(2, 0, 1, 3))
        gtb = np.ascontiguousarray(
            gate_proj[e].astype(_BF).reshape(KC, P, JC, P)
            .transpose(2, 1, 0, 3)).reshape(JC, P, DIN)
        dtb = np.ascontiguousarray(
            down_proj[e].astype(_BF).reshape(KC, P, JC, P)
            .transpose(2, 1, 0, 3)).reshape(JC, P, DIN)
        utb = np.ascontiguousarray(up_proj[e].astype(_BF)).reshape(JC, P, DIN)
        maps.append({"x_t": xtb, "gate_t": gtb, "down_t": dtb, "up_t": utb})
    return maps


_program = None


def kernel(x, gate_proj, down_proj, up_proj):
    global _program
    if _program is None:
        _program = build_program()
    in_maps = make_in_maps(
        np.asarray(x, dtype=np.float32),
        np.asarray(gate_proj, dtype=np.float32),
        np.asarray(down_proj, dtype=np.float32),
        np.asarray(up_proj, dtype=np.float32),
    )
    res = run_bass_kernel_spmd(_program, in_maps, list(range(E)))
    return np.stack([res.results[e]["out"] for e in range(E)], axis=0)


# revision 28
# speedup vs baseline: 1.0136x; 1.0136x over previous
"""Grouped-expert SwiGLU (MoE) kernel for Trainium2, expert-parallel over 8 cores.

Per core (one expert):
    g = x @ W_gate          [T, DOUT]
    u = x @ W_down          [T, DOUT]
    h = silu(g) * u
    out = h @ W_up          [T, DIN]

All inputs are pre-cast to bf16 and pre-laid-out on the host so the device
does no transposes and no input casts — the PE runs a dense LDW+MM stream at
the bf16 roofline (~216 ns per [128x128]x[128x512] matmul):
  x_t    [S1, KC, P, NS]  xT chunks: x_t[s,k,p,n] = x[s*NS+n, k*P+p]
  gate_t [JC, P, DIN]     per-j panels: gate_t[j,p,k*P+n] = Wg[k*P+p, j*P+n]
  down_t [JC, P, DIN]     same layout as gate_t
  up_t   [JC, P, DIN]     up_t[j,p,c] = Wu[j*P+p, c]
phase 1: hT[j] = silu(Wg[:,j].T @ xT) * (Wd[:,j].T @ xT)   [dout, tokens]
phase 2: out[m,:] = sum_j hT[j][:,m].T @ Wu[j,:]           [tokens, din]
Matmuls in bf16 with fp32 PSUM accumulation.

DMA notes (measured): each DMA_DIRECT2D dispatch costs ~650ns serialized on
its ring (Sync/Scalar are the two HWDGE rings); a single in-flight transfer
streams ~100GB/s, concurrent transfers ~400GB/s aggregate.  So the j0 window
emits DMAs in exact consumption order, finest pieces first, across both
rings; later strips use big transfers (fewer dispatches).
"""

import numpy as np
import ml_dtypes

import concourse.bacc as bacc
import concourse.mybir as mybir
from concourse.tile import TileContext
from concourse.bass_utils import run_bass_kernel_spmd

F32 = mybir.dt.float32
BF16 = mybir.dt.bfloat16
SILU = mybir.ActivationFunctionType.Silu
SIGMOID = mybir.ActivationFunctionType.Sigmoid
COPY = mybir.ActivationFunctionType.Copy

E = 8
T, DIN, DOUT = 2048, 2048, 1408
P = 128
NS = 512
KC = DIN // P   # 16 contraction chunks (din)
JC = DOUT // P  # 11 dout blocks
MC = T // P     # 16 token blocks
S1 = T // NS    # 4 token strips
S2 = DIN // NS  # 4 din strips


def build_program(sim_safe=False):
    nc = bacc.Bacc(target_bir_lowering=False, trn_type="TRN2")
    xt = nc.dram_tensor("x_t", [S1, KC, P, NS], BF16, kind="ExternalInput")
    wg = nc.dram_tensor("gate_t", [JC, P, DIN], BF16, kind="ExternalInput")
    wd = nc.dram_tensor("down_t", [JC, P, DIN], BF16, kind="ExternalInput")
    wu = nc.dram_tensor("up_t", [JC, P, DIN], BF16, kind="ExternalInput")
    out = nc.dram_tensor("out", [T, DIN], F32, kind="ExternalOutput")

    with TileContext(nc) as tc:
        with tc.tile_pool(name="persist", bufs=1) as persist:
            xts0 = [persist.tile([P, 2, NS], BF16, tag=f"xt0_{c}",
                                 name=f"xt0_{c}")
                    for c in range(KC // 2)]
            xts3 = [persist.tile([P, KC, NS], BF16, tag=f"xt3_{s}",
                                 name=f"xt3_{s}")
                    for s in range(1, S1)]
            hT = [persist.tile([P, T], BF16, tag=f"hT{j}", name=f"hT{j}")
                  for j in range(JC)]
            wub = [persist.tile([P, DIN], BF16, tag=f"wub{j}", name=f"wub{j}")
                   for j in range(JC)]

            with tc.tile_pool(name="wstage", bufs=2) as wstage, \
                 tc.tile_pool(name="silu", bufs=3) as silu_pool, \
                 tc.tile_pool(name="ostage", bufs=4) as ostage, \
                 tc.tile_pool(name="p1", bufs=2, space="PSUM") as p1, \
                 tc.tile_pool(name="p2", bufs=4, space="PSUM") as p2:

                def x_rhs(s, k):
                    if s == 0:
                        return xts0[k // 2][:, k % 2, :]
                    return xts3[s - 1][:, k, :]

                # HAM warmup: 8 dependency-free matmuls on a zeroed tile run
                # 6.0-9.4us (while the first input DMAs are in flight) so the
                # clock gate reaches 8/8 right as the real stream begins
                if not sim_safe:
                    warm = silu_pool.tile([P, NS], BF16, tag="sl", name="warm")
                    nc.vector.memset(warm, 0)
                    for i in range(8):
                        pw = p2.tile([P, NS], F32, tag="po", name=f"pw{i}")
                        nc.tensor.matmul(pw, lhsT=warm[:, 0:P], rhs=warm,
                                         start=True, stop=True)

                # ---- phase 1: hT[j] = silu(gT) * uT ----
                for j in range(JC):
                    wgp = wstage.tile([P, DIN], BF16, tag="wgp", name=f"wgp{j}")
                    wdp = wstage.tile([P, DIN], BF16, tag="wdp", name=f"wdp{j}")
                    if j == 0:
                        # j0 panels in 128KB pieces so the first matmul only
                        # waits on a piece of panel + one x chunk
                        for wp, wdr in ((wgp, wg), (wdp, wd)):
                            for q in range(4):
                                cols = slice(4 * q * P, 4 * (q + 1) * P)
                                nc.sync.dma_start(out=wp[:, cols],
                                                  in_=wdr.ap()[0][:, cols])
                        # strip 0: first pair as two 128KB singles, then
                        # 256KB chunks on the Scalar ring; the last three
                        # chunks go on the Sync ring (ahead of the strip
                        # quarters) so both rings finish strip 0 early
                        for k in range(2):
                            nc.scalar.dma_start(
                                out=xts0[0][:, k:k + 1, :],
                                in_=xt.ap()[0, k:k + 1]
                                .rearrange("k p n -> p k n"))
                        for c in range(1, KC // 2):
                            ring = nc.scalar if c < 5 else nc.sync
                            ring.dma_start(
                                out=xts0[c],
                                in_=xt.ap()[0, 2 * c:2 * c + 2]
                                .rearrange("k p n -> p k n"))
                        # strips 1-3 in 512KB quarters on the Sync ring
                        for s in range(1, S1):
                            for q in range(4):
                                nc.sync.dma_start(
                                    out=xts3[s - 1][:, 4 * q:4 * q + 4, :],
                                    in_=xt.ap()[s, 4 * q:4 * q + 4]
                                    .rearrange("k p n -> p k n"))
                    elif j == 1:
                        # j1 panels ride the otherwise-idle Scalar ring so
                        # the Sync ring stays dedicated to x strips
                        nc.scalar.dma_start(out=wgp, in_=wg.ap()[j])
                        nc.scalar.dma_start(out=wdp, in_=wd.ap()[j])
                    else:
                        nc.sync.dma_start(out=wgp, in_=wg.ap()[j])
                        nc.sync.dma_start(out=wdp, in_=wd.ap()[j])
                    for s in range(S1):
                        pg = p1.tile([P, NS], F32, tag="pg", name="pg")
                        pu = p1.tile([P, NS], F32, tag="pu", name="pu")
                        # interleave gate/down per k: halves the x-chunk
                        # consumption rate so DMA supply keeps up during j0
                        for k in range(KC):
                            nc.tensor.matmul(
                                pg, lhsT=wgp[:, k * P:(k + 1) * P],
                                rhs=x_rhs(s, k),
                                start=(k == 0), stop=(k == KC - 1))
                            nc.tensor.matmul(
                                pu, lhsT=wdp[:, k * P:(k + 1) * P],
                                rhs=x_rhs(s, k),
                                start=(k == 0), stop=(k == KC - 1))
                        sl = silu_pool.tile([P, NS], BF16, tag="sl", name="sl")
                        if sim_safe:
                            # CoreSim has no Silu; silu(g) = g * sigmoid(g)
                            nc.scalar.activation(sl, pg, SIGMOID)
                            nc.vector.tensor_mul(out=sl, in0=sl, in1=pg)
                        else:
                            nc.scalar.activation(sl, pg, SILU)
                        nc.vector.tensor_mul(out=hT[j][:, s * NS:(s + 1) * NS],
                                             in0=sl, in1=pu)

                # stage phase-2 weights; the Sync ring reaches these right
                # after the phase-1 panels, well before phase 2 needs them
                for j in range(JC):
                    nc.sync.dma_start(out=wub[j], in_=wu.ap()[j])

                # ---- phase 2: out = hT.T @ Wu ----
                for m in range(MC):
                    for n in range(S2):
                        msl = slice(m * P, (m + 1) * P)
                        if m == MC - 1 and n == S2 - 1:
                            # final group as two N=256 halves: the first
                            # half's evict+DMA overlaps the second half's
                            # matmuls, shortening the kernel tail
                            H = NS // 2
                            for h in range(2):
                                dsl = slice(n * NS + h * H,
                                            n * NS + (h + 1) * H)
                                po = p2.tile([P, H], F32, tag="po",
                                             name=f"poL{h}")
                                for j in range(JC):
                                    nc.tensor.matmul(
                                        po, lhsT=hT[j][:, msl],
                                        rhs=wub[j][:, dsl],
                                        start=(j == 0), stop=(j == JC - 1))
                                ot = ostage.tile([P, H], F32, tag="ot",
                                                 name=f"oL{h}")
                                if h == 0:
                                    nc.vector.tensor_copy(out=ot, in_=po)
                                    nc.sync.dma_start(
                                        out=out.ap()[msl, dsl], in_=ot)
                                else:
                                    nc.scalar.activation(ot, po, COPY)
                                    nc.scalar.dma_start(
                                        out=out.ap()[msl, dsl], in_=ot)
                            continue
                        dsl = slice(n * NS, (n + 1) * NS)
                        po = p2.tile([P, NS], F32, tag="po", name="po")
                        for j in range(JC):
                            nc.tensor.matmul(
                                po, lhsT=hT[j][:, msl],
                                rhs=wub[j][:, dsl],
                                start=(j == 0), stop=(j == JC - 1))
                        ot = ostage.tile([P, NS], F32, tag="ot", name="ot")
                        if (m * S2 + n) % 2 == 0:
                            nc.scalar.activation(ot, po, COPY)
                        else:
                            nc.vector.tensor_copy(out=ot, in_=po)
                        nc.sync.dma_start(
                            out=out.ap()[msl, dsl], in_=ot)

    nc.finalize()
    return nc


_BF = ml_dtypes.bfloat16


def make_in_maps(x, gate_proj, down_proj, up_proj):
    maps = []
    for e in range(E):
        xtb = x[e].T.astype(_BF)  # [DIN, T]
        xtb = np.ascontiguousarray(
            xtb.reshape(KC, P, S1, NS).transpose(2, 0, 1, 3))
        gtb = np.ascontiguousarray(
            gate_proj[e].astype(_BF).reshape(KC, P, JC, P)
            .transpose(2, 1, 0, 3)).reshape(JC, P, DIN)
        dtb = np.ascontiguousarray(
            down_proj[e].astype(_BF).reshape(KC, P, JC, P)
            .transpose(2, 1, 0, 3)).reshape(JC, P, DIN)
        utb = np.ascontiguousarray(up_proj[e].astype(_BF)).reshape(JC, P, DIN)
        maps.append({"x_t": xtb, "gate_t": gtb, "down_t": dtb, "up_t": utb})
    return maps


_program = None


def kernel(x, gate_proj, down_proj, up_proj):
    global _program
    if _program is None:
        _program = build_program()
    in_maps = make_in_maps(
        np.asarray(x, dtype=np.float32),
        np.asarray(gate_proj, dtype=np.float32),
        np.asarray(down_proj, dtype=np.float32),
        np.asarray(up_proj, dtype=np.float32),
    )
    res = run_bass_kernel_spmd(_program, in_maps, list(range(E)))
    return np.stack([res.results[e]["out"] for e in range(E)], axis=0)


# revision 34
# speedup vs baseline: 1.0148x; 1.0012x over previous
"""Grouped-expert SwiGLU (MoE) kernel for Trainium2, expert-parallel over 8 cores.

Per core (one expert):
    g = x @ W_gate          [T, DOUT]
    u = x @ W_down          [T, DOUT]
    h = silu(g) * u
    out = h @ W_up          [T, DIN]

All inputs are pre-cast to bf16 and pre-laid-out on the host so the device
does no transposes and no input casts — the PE runs a dense LDW+MM stream at
the bf16 roofline (~216 ns per [128x128]x[128x512] matmul):
  x_t    [S1, KC, P, NS]  xT chunks: x_t[s,k,p,n] = x[s*NS+n, k*P+p]
  gate_t [JC, P, DIN]     per-j panels: gate_t[j,p,k*P+n] = Wg[k*P+p, j*P+n]
  down_t [JC, P, DIN]     same layout as gate_t
  up_t   [JC, P, DIN]     up_t[j,p,c] = Wu[j*P+p, c]
phase 1: hT[j] = silu(Wg[:,j].T @ xT) * (Wd[:,j].T @ xT)   [dout, tokens]
phase 2: out[m,:] = sum_j hT[j][:,m].T @ Wu[j,:]           [tokens, din]
Matmuls in bf16 with fp32 PSUM accumulation.

DMA notes (measured): each DMA_DIRECT2D dispatch costs ~650ns serialized on
its ring (Sync/Scalar are the two HWDGE rings); a single in-flight transfer
streams ~100GB/s, concurrent transfers ~400GB/s aggregate.  So the j0 window
emits DMAs in exact consumption order, finest pieces first, across both
rings; later strips use big transfers (fewer dispatches).
"""

import numpy as np
import ml_dtypes

import concourse.bacc as bacc
import concourse.mybir as mybir
from concourse.tile import TileContext
from concourse.bass_utils import run_bass_kernel_spmd

F32 = mybir.dt.float32
BF16 = mybir.dt.bfloat16
SILU = mybir.ActivationFunctionType.Silu
SIGMOID = mybir.ActivationFunctionType.Sigmoid
COPY = mybir.ActivationFunctionType.Copy

E = 8
T, DIN, DOUT = 2048, 2048, 1408
P = 128
NS = 512
KC = DIN // P   # 16 contraction chunks (din)
JC = DOUT // P  # 11 dout blocks
MC = T // P     # 16 token blocks
S1 = T // NS    # 4 token strips
S2 = DIN // NS  # 4 din strips


def build_program(sim_safe=False):
    nc = bacc.Bacc(target_bir_lowering=False, trn_type="TRN2")
    # quad-fused x layout: x_t[s, q, p, :] holds k-chunks 4q..4q+3 for
    # partition p as one 4KB-contiguous run, so strip DMAs use 4KB
    # descriptors instead of 1KB (higher per-stream DMA bandwidth)
    xt = nc.dram_tensor("x_t", [S1, KC // 4, P, 4 * NS], BF16,
                        kind="ExternalInput")
    wg = nc.dram_tensor("gate_t", [JC, P, DIN], BF16, kind="ExternalInput")
    wd = nc.dram_tensor("down_t", [JC, P, DIN], BF16, kind="ExternalInput")
    wu = nc.dram_tensor("up_t", [JC, P, DIN], BF16, kind="ExternalInput")
    out = nc.dram_tensor("out", [T, DIN], F32, kind="ExternalOutput")

    with TileContext(nc) as tc:
        with tc.tile_pool(name="persist", bufs=1) as persist:
            xts0 = [persist.tile([P, 4, NS], BF16, tag=f"xt0_{q}",
                                 name=f"xt0_{q}")
                    for q in range(KC // 4)]
            xts3 = [persist.tile([P, KC, NS], BF16, tag=f"xt3_{s}",
                                 name=f"xt3_{s}")
                    for s in range(1, S1)]
            hT = [persist.tile([P, T], BF16, tag=f"hT{j}", name=f"hT{j}")
                  for j in range(JC)]
            wub = [persist.tile([P, DIN], BF16, tag=f"wub{j}", name=f"wub{j}")
                   for j in range(JC)]

            with tc.tile_pool(name="wstage", bufs=2) as wstage, \
                 tc.tile_pool(name="silu", bufs=3) as silu_pool, \
                 tc.tile_pool(name="ostage", bufs=4) as ostage, \
                 tc.tile_pool(name="p1", bufs=2, space="PSUM") as p1, \
                 tc.tile_pool(name="p2", bufs=4, space="PSUM") as p2:

                def x_rhs(s, k):
                    if s == 0:
                        return xts0[k // 4][:, k % 4, :]
                    return xts3[s - 1][:, k, :]



                # ---- phase 1: hT[j] = silu(gT) * uT ----
                for j in range(JC):
                    wgp = wstage.tile([P, DIN], BF16, tag="wgp", name=f"wgp{j}")
                    wdp = wstage.tile([P, DIN], BF16, tag="wdp", name=f"wdp{j}")
                    if j == 0:
                        # j0 panels in 128KB pieces so the first matmul only
                        # waits on a piece of panel + one x chunk
                        for wp, wdr in ((wgp, wg), (wdp, wd)):
                            for q in range(4):
                                cols = slice(4 * q * P, 4 * (q + 1) * P)
                                nc.sync.dma_start(out=wp[:, cols],
                                                  in_=wdr.ap()[0][:, cols])
                        # strip 0: two 128KB singles, then 256KB halves on
                        # the Scalar ring; the last chunks go on the Sync
                        # ring (ahead of the strip quarters) so both rings
                        # finish strip 0 early.  Pieces index into the
                        # quad-contiguous DRAM runs.
                        for k in range(2):
                            nc.scalar.dma_start(
                                out=xts0[0][:, k, :],
                                in_=xt.ap()[0, 0][:, k * NS:(k + 1) * NS])
                        nc.scalar.dma_start(
                            out=xts0[0][:, 2:4, :],
                            in_=xt.ap()[0, 0][:, 2 * NS:4 * NS]
                            .rearrange("p (k n) -> p k n", n=NS))
                        for c in range(2, KC // 2):
                            q, h = c // 2, c % 2
                            ring = nc.scalar if c < 5 else nc.sync
                            ring.dma_start(
                                out=xts0[q][:, 2 * h:2 * h + 2, :],
                                in_=xt.ap()[0, q][:, 2 * h * NS:
                                                  (2 * h + 2) * NS]
                                .rearrange("p (k n) -> p k n", n=NS))
                        # strips 1-3 in 512KB quarters (4KB descriptors)
                        # on the Sync ring
                        for s in range(1, S1):
                            for q in range(4):
                                nc.sync.dma_start(
                                    out=xts3[s - 1][:, 4 * q:4 * q + 4, :],
                                    in_=xt.ap()[s, q]
                                    .rearrange("p (k n) -> p k n", n=NS))
                    elif j == 1:
                        # j1 panels ride the otherwise-idle Scalar ring so
                        # the Sync ring stays dedicated to x strips
                        nc.scalar.dma_start(out=wgp, in_=wg.ap()[j])
                        nc.scalar.dma_start(out=wdp, in_=wd.ap()[j])
                    else:
                        nc.sync.dma_start(out=wgp, in_=wg.ap()[j])
                        nc.sync.dma_start(out=wdp, in_=wd.ap()[j])
                    for s in range(S1):
                        pg = p1.tile([P, NS], F32, tag="pg", name="pg")
                        pu = p1.tile([P, NS], F32, tag="pu", name="pu")
                        # interleave gate/down per k: halves the x-chunk
                        # consumption rate so DMA supply keeps up during j0
                        for k in range(KC):
                            nc.tensor.matmul(
                                pg, lhsT=wgp[:, k * P:(k + 1) * P],
                                rhs=x_rhs(s, k),
                                start=(k == 0), stop=(k == KC - 1))
                            nc.tensor.matmul(
                                pu, lhsT=wdp[:, k * P:(k + 1) * P],
                                rhs=x_rhs(s, k),
                                start=(k == 0), stop=(k == KC - 1))
                        sl = silu_pool.tile([P, NS], BF16, tag="sl", name="sl")
                        if sim_safe:
                            # CoreSim has no Silu; silu(g) = g * sigmoid(g)
                            nc.scalar.activation(sl, pg, SIGMOID)
                            nc.vector.tensor_mul(out=sl, in0=sl, in1=pg)
                        else:
                            nc.scalar.activation(sl, pg, SILU)
                        nc.vector.tensor_mul(out=hT[j][:, s * NS:(s + 1) * NS],
                                             in0=sl, in1=pu)

                # stage phase-2 weights; the Sync ring reaches these right
                # after the phase-1 panels, well before phase 2 needs them
                for j in range(JC):
                    nc.sync.dma_start(out=wub[j], in_=wu.ap()[j])

                # ---- phase 2: out = hT.T @ Wu ----
                for m in range(MC):
                    for n in range(S2):
                        msl = slice(m * P, (m + 1) * P)
                        if m == MC - 1 and n == S2 - 1:
                            # final group as two N=256 halves: the first
                            # half's evict+DMA overlaps the second half's
                            # matmuls, shortening the kernel tail
                            H = NS // 2
                            for h in range(2):
                                dsl = slice(n * NS + h * H,
                                            n * NS + (h + 1) * H)
                                po = p2.tile([P, H], F32, tag="po",
                                             name=f"poL{h}")
                                for j in range(JC):
                                    nc.tensor.matmul(
                                        po, lhsT=hT[j][:, msl],
                                        rhs=wub[j][:, dsl],
                                        start=(j == 0), stop=(j == JC - 1))
                                ot = ostage.tile([P, H], F32, tag="ot",
                                                 name=f"oL{h}")
                                if h == 0:
                                    nc.vector.tensor_copy(out=ot, in_=po)
                                    nc.sync.dma_start(
                                        out=out.ap()[msl, dsl], in_=ot)
                                else:
                                    nc.scalar.activation(ot, po, COPY)
                                    nc.scalar.dma_start(
                                        out=out.ap()[msl, dsl], in_=ot)
                            continue
                        dsl = slice(n * NS, (n + 1) * NS)
                        po = p2.tile([P, NS], F32, tag="po", name="po")
                        for j in range(JC):
                            nc.tensor.matmul(
                                po, lhsT=hT[j][:, msl],
                                rhs=wub[j][:, dsl],
                                start=(j == 0), stop=(j == JC - 1))
                        ot = ostage.tile([P, NS], F32, tag="ot", name="ot")
                        if (m * S2 + n) % 2 == 0:
                            nc.scalar.activation(ot, po, COPY)
                        else:
                            nc.vector.tensor_copy(out=ot, in_=po)
                        nc.sync.dma_start(
                            out=out.ap()[msl, dsl], in_=ot)

    nc.finalize()
    return nc


_BF = ml_dtypes.bfloat16


def make_in_maps(x, gate_proj, down_proj, up_proj):
    maps = []
    for e in range(E):
        xtb = x[e].T.astype(_BF)  # [DIN, T]
        # [S1, KC//4, P, 4, NS]: per-(s,q,p) the 4 k-chunk rows are fused
        # into one contiguous 4KB run
        xtb = np.ascontiguousarray(
            xtb.reshape(KC // 4, 4, P, S1, NS).transpose(3, 0, 2, 1, 4)
        ).reshape(S1, KC // 4, P, 4 * NS)
        gtb = np.ascontiguousarray(
            gate_proj[e].astype(_BF).reshape(KC, P, JC, P)
            .transpose(2, 1, 0, 3)).reshape(JC, P, DIN)
        dtb = np.ascontiguousarray(
            down_proj[e].astype(_BF).reshape(KC, P, JC, P)
            .transpose(2, 1, 0, 3)).reshape(JC, P, DIN)
        utb = np.ascontiguousarray(up_proj[e].astype(_BF)).reshape(JC, P, DIN)
        maps.append({"x_t": xtb, "gate_t": gtb, "down_t": dtb, "up_t": utb})
    return maps


_program = None


def kernel(x, gate_proj, down_proj, up_proj):
    global _program
    if _program is None:
        _program = build_program()
    in_maps = make_in_maps(
        np.asarray(x, dtype=np.float32),
        np.asarray(gate_proj, dtype=np.float32),
        np.asarray(down_proj, dtype=np.float32),
        np.asarray(up_proj, dtype=np.float32),
    )
    res = run_bass_kernel_spmd(_program, in_maps, list(range(E)))
    return np.stack([res.results[e]["out"] for e in range(E)], axis=0)
